# Initial kernel scaffold
#
"""Content-based (additive / Bahdanau) attention kernel for Trainium2.

Math (per batch element, one NeuronCore each — pure data parallel over B=8):
    fc_dec = dec @ W_dec.T + b_dec                    # (D, F)
    fc_enc = enc @ W_enc.T + b_enc                    # (E, F)
    scores[d, e] = sum_f v[f] * tanh(fc_dec[d, f] + fc_enc[e, f])
    attn = softmax_e(scores)                          # (D, E)
    context = attn @ enc                              # (D, F)

Layout strategy:
  * F lives on SBUF partitions (2 chunks of 128) so the per-d broadcast add
    is a DVE tensor_scalar (2x mode) and the f-reduction is a PE matmul
    with v as the stationary operand.
  * tanh runs on ACT in [128, G*512] batches (G=8 decoder rows per
    instruction) to amortize the ~224-cycle ScalarE instruction overhead.
  * The score matmul pads v into column r of a [128, 32] stationary tile so
    row r of the 32-row PSUM region receives the scores for decoder row
    d = 32*j + r (tile_position=(0, 32j)); the other 31 rows accumulate
    exact zeros.  float32r streams 1 row/cycle (fp32 would be 4x slower).
  * Softmax over e is done on [128 d, 512 e] PSUM tiles; context matmul
    contracts over e with PE-transposed attention blocks.
"""

from contextlib import ExitStack

import numpy as np

import concourse.bacc as bacc
import concourse.bass as bass
import concourse.mybir as mybir
import concourse.tile as tile
from concourse.bass_utils import run_bass_kernel_spmd
from concourse.masks import make_identity

F32 = mybir.dt.float32
F32R = mybir.dt.float32r

B, D, E, F = 8, 256, 512, 256
N_CORES = 8
G = 8  # decoder rows per tanh batch
DBLK = 128  # decoder rows per softmax block
N_GROUPS = DBLK // G  # 16 groups per block
FC = F // 128  # 2 f-chunks
EC = E // 128  # 4 e-chunks

_AF = mybir.ActivationFunctionType


def _build_nc():
    nc = bacc.Bacc()

    dec = nc.dram_tensor("decoder_states", [D, F], F32, kind="ExternalInput")
    enc = nc.dram_tensor("encoder_states", [E, F], F32, kind="ExternalInput")
    w_enc = nc.dram_tensor("W_enc", [F, F], F32, kind="ExternalInput")
    b_enc = nc.dram_tensor("b_enc", [F], F32, kind="ExternalInput")
    w_dec = nc.dram_tensor("W_dec", [F, F], F32, kind="ExternalInput")
    b_dec = nc.dram_tensor("b_dec", [F], F32, kind="ExternalInput")
    v = nc.dram_tensor("v", [F], F32, kind="ExternalInput")
    ctx_o = nc.dram_tensor("context", [D, F], F32, kind="ExternalOutput")
    attn_o = nc.dram_tensor("attn", [D, E], F32, kind="ExternalOutput")

    with tile.TileContext(nc) as tc:
        with ExitStack() as es:
            consts = es.enter_context(tc.tile_pool(name="consts", bufs=1))
            ld = es.enter_context(tc.tile_pool(name="ld", bufs=1))
            xp = es.enter_context(tc.tile_pool(name="xp", bufs=2))
            sm = es.enter_context(tc.tile_pool(name="sm", bufs=2))
            ptr = es.enter_context(tc.tile_pool(name="ptr", bufs=2, space="PSUM"))
            ps = es.enter_context(tc.tile_pool(name="ps", bufs=2, space="PSUM"))
            pctx = es.enter_context(tc.tile_pool(name="pctx", bufs=2, space="PSUM"))

            ident = consts.tile([128, 128], F32, name="ident")
            make_identity(nc, ident)

            # ---- raw input loads ----
            enc_nat = []
            for p in range(EC):
                t_ = consts.tile([128, F], F32, name=f"enc_nat{p}")
                nc.sync.dma_start(out=t_, in_=enc[p * 128 : (p + 1) * 128, :])
                enc_nat.append(t_)
            dec_nat = []
            for p in range(D // 128):
                t_ = ld.tile([128, F], F32, name=f"dec_nat{p}", tag=f"dec_nat{p}")
                nc.sync.dma_start(out=t_, in_=dec[p * 128 : (p + 1) * 128, :])
                dec_nat.append(t_)
            wenc_nat = []
            wdec_nat = []
            for p in range(FC):
                t_ = ld.tile([128, F], F32, name=f"wenc_nat{p}", tag=f"wenc_nat{p}")
                nc.sync.dma_start(out=t_, in_=w_enc[p * 128 : (p + 1) * 128, :])
                wenc_nat.append(t_)
                t2 = ld.tile([128, F], F32, name=f"wdec_nat{p}", tag=f"wdec_nat{p}")
                nc.sync.dma_start(out=t2, in_=w_dec[p * 128 : (p + 1) * 128, :])
                wdec_nat.append(t2)
            b_enc_sb = []
            b_dec_sb = []
            v_sb = []
            b_enc_r = b_enc[:].rearrange("(c p one) -> c p one", c=FC, one=1)
            b_dec_r = b_dec[:].rearrange("(c p one) -> c p one", c=FC, one=1)
            v_r = v[:].rearrange("(c p one) -> c p one", c=FC, one=1)
            for c in range(FC):
                tb = consts.tile([128, 1], F32, name=f"b_enc_sb{c}")
                nc.sync.dma_start(out=tb, in_=b_enc_r[c])
                b_enc_sb.append(tb)
                tb2 = consts.tile([128, 1], F32, name=f"b_dec_sb{c}")
                nc.sync.dma_start(out=tb2, in_=b_dec_r[c])
                b_dec_sb.append(tb2)
                tv = consts.tile([128, 1], F32, name=f"v_sb{c}")
                nc.sync.dma_start(out=tv, in_=v_r[c])
                v_sb.append(tv)

            # ---- transposes via PE ----
            def pe_t(dst, src):
                pt = ptr.tile([128, 128], F32, name="pt", tag="pt")
                nc.tensor.transpose(pt, src, ident)
                nc.vector.tensor_copy(out=dst, in_=pt)

            # W_enc_T[j] = [f_in chunk j on partitions, f_out free]
            w_enc_t = [consts.tile([128, F], F32, name=f"w_enc_t{j}") for j in range(FC)]
            w_dec_t = [consts.tile([128, F], F32, name=f"w_dec_t{j}") for j in range(FC)]
            for i in range(FC):
                for j in range(FC):
                    pe_t(w_enc_t[j][:, i * 128 : (i + 1) * 128],
                         wenc_nat[i][:, j * 128 : (j + 1) * 128])
                    pe_t(w_dec_t[j][:, i * 128 : (i + 1) * 128],
                         wdec_nat[i][:, j * 128 : (j + 1) * 128])
            # enc_T[c] = [f chunk c on partitions, e free]
            enc_t = [consts.tile([128, E], F32, name=f"enc_t{c}") for c in range(FC)]
            for p in range(EC):
                for c in range(FC):
                    pe_t(enc_t[c][:, p * 128 : (p + 1) * 128],
                         enc_nat[p][:, c * 128 : (c + 1) * 128])
            dec_t = [consts.tile([128, D], F32, name=f"dec_t{c}") for c in range(FC)]
            for p in range(D // 128):
                for c in range(FC):
                    pe_t(dec_t[c][:, p * 128 : (p + 1) * 128],
                         dec_nat[p][:, c * 128 : (c + 1) * 128])

            # ---- fc projections (full fp32 precision) ----
            # fc_enc_T[m] = [f_out chunk m, e] = W_enc @ enc.T + b_enc
            fc_enc_t = [consts.tile([128, E], F32, name=f"fc_enc_t{m}") for m in range(FC)]
            fc_dec_t = [consts.tile([128, D], F32, name=f"fc_dec_t{m}") for m in range(FC)]
            for m in range(FC):
                pf = ps.tile([128, E], F32, name="pf", tag="ps")
                for k in range(FC):
                    nc.tensor.matmul(
                        pf,
                        lhsT=w_enc_t[k][:, m * 128 : (m + 1) * 128],
                        rhs=enc_t[k],
                        start=(k == 0),
                        stop=(k == FC - 1),
                    )
                nc.vector.tensor_scalar_add(out=fc_enc_t[m], in0=pf, scalar1=b_enc_sb[m])
            for m in range(FC):
                pf = ps.tile([128, D], F32, name="pf2", tag="ps")
                for k in range(FC):
                    nc.tensor.matmul(
                        pf,
                        lhsT=w_dec_t[k][:, m * 128 : (m + 1) * 128],
                        rhs=dec_t[k],
                        start=(k == 0),
                        stop=(k == FC - 1),
                    )
                nc.vector.tensor_scalar_add(out=fc_dec_t[m], in0=pf, scalar1=b_dec_sb[m])

            # ---- v stationary tiles: 32 tiles of [128, 32], v in column r ----
            # tile r occupies cols [r*32, r*32+32); its column r is abs col r*33
            vw = []
            for c in range(FC):
                vt = consts.tile([128, 1024], F32, name=f"vw{c}")
                nc.vector.memset(vt, 0.0)
                nc.vector.tensor_copy(
                    out=vt[:, 0:1024:33], in_=v_sb[c].to_broadcast([128, 32])
                )
                vw.append(vt)

            # ---- main loop ----
            for blk in range(D // DBLK):
                pscore = ps.tile([128, E], F32, name="pscore", tag="ps")
                for g in range(N_GROUPS):
                    xg = [
                        xp.tile([128, G * E], F32, name=f"x{c}", tag=f"x{c}")
                        for c in range(FC)
                    ]
                    for r in range(G):
                        dg = blk * DBLK + g * G + r
                        for c in range(FC):
                            nc.vector.tensor_scalar_add(
                                out=xg[c][:, r * E : (r + 1) * E],
                                in0=fc_enc_t[c],
                                scalar1=fc_dec_t[c][:, dg : dg + 1],
                            )
                    for c in range(FC):
                        nc.scalar.activation(out=xg[c], in_=xg[c], func=_AF.Tanh)
                    for r in range(G):
                        dl = g * G + r
                        j, r32 = dl // 32, dl % 32
                        for c in range(FC):
                            nc.tensor.matmul(
                                pscore[j * 32 : (j + 1) * 32, :],
                                lhsT=vw[c][:, r32 * 32 : (r32 + 1) * 32].bitcast(F32R),
                                rhs=xg[c][:, r * E : (r + 1) * E].bitcast(F32R),
                                start=(dl % 32 == 0 and c == 0),
                                stop=(dl % 32 == 31 and c == FC - 1),
                                tile_position=(0, j * 32),
                            )
                # softmax over e for the 128 decoder rows of this block
                negmx = sm.tile([128, 1], F32, name="negmx")
                nc.vector.reduce_max(
                    out=negmx, in_=pscore, axis=mybir.AxisListType.X, negate=True
                )
                et = sm.tile([128, E], F32, name="et")
                nc.scalar.activation(out=et, in_=pscore, func=_AF.Exp, bias=negmx)
                ssum = sm.tile([128, 1], F32, name="ssum")
                nc.vector.reduce_sum(out=ssum, in_=et, axis=mybir.AxisListType.X)
                rs = sm.tile([128, 1], F32, name="rs")
                nc.vector.reciprocal(rs, ssum)
                attn_sb = sm.tile([128, E], F32, name="attn_sb")
                nc.vector.tensor_scalar_mul(out=attn_sb, in0=et, scalar1=rs)
                nc.sync.dma_start(
                    out=attn_o[blk * DBLK : (blk + 1) * DBLK, :], in_=attn_sb
                )
                # context = attn @ enc, contraction over e
                at_tiles = []
                for k in range(EC):
                    pt2 = ptr.tile([128, 128], F32, name="pt2", tag="pt")
                    nc.tensor.transpose(pt2, attn_sb[:, k * 128 : (k + 1) * 128], ident)
                    at = sm.tile([128, 128], F32, name="at", tag="at")
                    nc.vector.tensor_copy(out=at, in_=pt2)
                    at_tiles.append(at)
                pc = pctx.tile([128, F], F32, name="pc")
                for k in range(EC):
                    nc.tensor.matmul(
                        pc,
                        lhsT=at_tiles[k],
                        rhs=enc_nat[k],
                        start=(k == 0),
                        stop=(k == EC - 1),
                    )
                ctx_sb = sm.tile([128, F], F32, name="ctx_sb")
                nc.vector.tensor_copy(out=ctx_sb, in_=pc)
                nc.sync.dma_start(
                    out=ctx_o[blk * DBLK : (blk + 1) * DBLK, :], in_=ctx_sb
                )

    nc.compile()
    return nc


_NC = None


def _get_nc():
    global _NC
    if _NC is None:
        _NC = _build_nc()
    return _NC


def _in_maps(inputs):
    dec = np.asarray(inputs["decoder_states"], dtype=np.float32)
    enc = np.asarray(inputs["encoder_states"], dtype=np.float32)
    w_enc = np.asarray(inputs["W_enc"], dtype=np.float32)
    b_enc = np.asarray(inputs["b_enc"], dtype=np.float32)
    w_dec = np.asarray(inputs["W_dec"], dtype=np.float32)
    b_dec = np.asarray(inputs["b_dec"], dtype=np.float32)
    v = np.asarray(inputs["v"], dtype=np.float32)
    return [
        {
            "decoder_states": np.ascontiguousarray(dec[b]),
            "encoder_states": np.ascontiguousarray(enc[b]),
            "W_enc": w_enc,
            "b_enc": b_enc,
            "W_dec": w_dec,
            "b_dec": b_dec,
            "v": v,
        }
        for b in range(B)
    ]


def kernel(**inputs):
    nc = _get_nc()
    res = run_bass_kernel_spmd(
        nc, _in_maps(inputs), core_ids=list(range(N_CORES))
    ).results
    context = np.stack([r["context"] for r in res], axis=0)
    attn = np.stack([r["attn"] for r in res], axis=0)
    return context, attn


# revision 10
# speedup vs baseline: 34.7304x; 34.7304x over previous
"""Content-based (additive / Bahdanau) attention kernel for Trainium2.

Math (per batch element, one NeuronCore each — pure data parallel over B=8):
    fc_dec = dec @ W_dec.T + b_dec                    # (D, F)
    fc_enc = enc @ W_enc.T + b_enc                    # (E, F)
    scores[d, e] = sum_f v[f] * tanh(fc_dec[d, f] + fc_enc[e, f])
    attn = softmax_e(scores)                          # (D, E)
    context = attn @ enc                              # (D, F)

Layout strategy:
  * F lives on SBUF partitions (2 chunks of 128) so the per-d broadcast add
    is a DVE tensor_scalar (2x mode) and the f-reduction is a PE matmul
    with v as the stationary operand.
  * tanh runs on ACT in [128, G*512] batches (G=8 decoder rows per
    instruction) to amortize the ~224-cycle ScalarE instruction overhead.
  * The score matmul pads v into column r of a [128, 32] stationary tile so
    row r of the 32-row PSUM region receives the scores for decoder row
    d = 32*j + r (tile_position=(0, 32j)); the other 31 rows accumulate
    exact zeros.  float32r streams 1 row/cycle (fp32 would be 4x slower).
  * Softmax over e is done on [128 d, 512 e] PSUM tiles; context matmul
    contracts over e with PE-transposed attention blocks.
"""

from contextlib import ExitStack

import numpy as np

import concourse.bacc as bacc
import concourse.bass as bass
import concourse.mybir as mybir
import concourse.tile as tile
from concourse.bass_utils import run_bass_kernel_spmd
from concourse.masks import make_identity

F32 = mybir.dt.float32
F32R = mybir.dt.float32r

B, D, E, F = 8, 256, 512, 256
N_CORES = 8
G = 8  # decoder rows per tanh batch
DBLK = 128  # decoder rows per softmax block
N_GROUPS = DBLK // G  # 16 groups per block
FC = F // 128  # 2 f-chunks
EC = E // 128  # 4 e-chunks

_AF = mybir.ActivationFunctionType


def _build_nc(reps=1):
    """reps>1 unrolls the main loop for steady-state benchmarking."""
    nc = bacc.Bacc()

    dec = nc.dram_tensor("decoder_states", [D, F], F32, kind="ExternalInput")
    enc = nc.dram_tensor("encoder_states", [E, F], F32, kind="ExternalInput")
    w_enc = nc.dram_tensor("W_enc", [F, F], F32, kind="ExternalInput")
    b_enc = nc.dram_tensor("b_enc", [F], F32, kind="ExternalInput")
    w_dec = nc.dram_tensor("W_dec", [F, F], F32, kind="ExternalInput")
    b_dec = nc.dram_tensor("b_dec", [F], F32, kind="ExternalInput")
    v = nc.dram_tensor("v", [F], F32, kind="ExternalInput")
    ctx_o = nc.dram_tensor("context", [D, F], F32, kind="ExternalOutput")
    attn_o = nc.dram_tensor("attn", [D, E], F32, kind="ExternalOutput")

    with tile.TileContext(nc) as tc:
        with ExitStack() as es:
            consts = es.enter_context(tc.tile_pool(name="consts", bufs=1))
            ld = es.enter_context(tc.tile_pool(name="ld", bufs=1))
            xp = es.enter_context(tc.tile_pool(name="xp", bufs=2))
            sm = es.enter_context(tc.tile_pool(name="sm", bufs=2))
            ptr = es.enter_context(tc.tile_pool(name="ptr", bufs=2, space="PSUM"))
            ps = es.enter_context(tc.tile_pool(name="ps", bufs=1, space="PSUM"))
            pctx = es.enter_context(tc.tile_pool(name="pctx", bufs=1, space="PSUM"))

            ident = consts.tile([128, 128], F32, name="ident")
            make_identity(nc, ident)

            # ---- raw input loads ----
            enc_nat = []
            for p in range(EC):
                t_ = consts.tile([128, F], F32, name=f"enc_nat{p}")
                nc.sync.dma_start(out=t_, in_=enc[p * 128 : (p + 1) * 128, :])
                enc_nat.append(t_)
            dec_nat = []
            for p in range(D // 128):
                t_ = ld.tile([128, F], F32, name=f"dec_nat{p}", tag=f"dec_nat{p}")
                nc.sync.dma_start(out=t_, in_=dec[p * 128 : (p + 1) * 128, :])
                dec_nat.append(t_)
            wenc_nat = []
            wdec_nat = []
            for p in range(FC):
                t_ = ld.tile([128, F], F32, name=f"wenc_nat{p}", tag=f"wenc_nat{p}")
                nc.sync.dma_start(out=t_, in_=w_enc[p * 128 : (p + 1) * 128, :])
                wenc_nat.append(t_)
                t2 = ld.tile([128, F], F32, name=f"wdec_nat{p}", tag=f"wdec_nat{p}")
                nc.sync.dma_start(out=t2, in_=w_dec[p * 128 : (p + 1) * 128, :])
                wdec_nat.append(t2)
            b_enc_sb = []
            b_dec_sb = []
            v_sb = []
            b_enc_r = b_enc[:].rearrange("(c p one) -> c p one", c=FC, one=1)
            b_dec_r = b_dec[:].rearrange("(c p one) -> c p one", c=FC, one=1)
            v_r = v[:].rearrange("(c p one) -> c p one", c=FC, one=1)
            for c in range(FC):
                tb = consts.tile([128, 1], F32, name=f"b_enc_sb{c}")
                nc.sync.dma_start(out=tb, in_=b_enc_r[c])
                b_enc_sb.append(tb)
                tb2 = consts.tile([128, 1], F32, name=f"b_dec_sb{c}")
                nc.sync.dma_start(out=tb2, in_=b_dec_r[c])
                b_dec_sb.append(tb2)
                tv = consts.tile([128, 1], F32, name=f"v_sb{c}")
                nc.sync.dma_start(out=tv, in_=v_r[c])
                v_sb.append(tv)

            # ---- transposes via PE ----
            def pe_t(dst, src):
                pt = ptr.tile([128, 128], F32, name="pt", tag="pt")
                nc.tensor.transpose(pt, src, ident)
                nc.vector.tensor_copy(out=dst, in_=pt)

            # W_enc_T[j] = [f_in chunk j on partitions, f_out free]
            w_enc_t = [consts.tile([128, F], F32, name=f"w_enc_t{j}") for j in range(FC)]
            w_dec_t = [consts.tile([128, F], F32, name=f"w_dec_t{j}") for j in range(FC)]
            for i in range(FC):
                for j in range(FC):
                    pe_t(w_enc_t[j][:, i * 128 : (i + 1) * 128],
                         wenc_nat[i][:, j * 128 : (j + 1) * 128])
                    pe_t(w_dec_t[j][:, i * 128 : (i + 1) * 128],
                         wdec_nat[i][:, j * 128 : (j + 1) * 128])
            # enc_T[c] = [f chunk c on partitions, e free]
            enc_t = [consts.tile([128, E], F32, name=f"enc_t{c}") for c in range(FC)]
            for p in range(EC):
                for c in range(FC):
                    pe_t(enc_t[c][:, p * 128 : (p + 1) * 128],
                         enc_nat[p][:, c * 128 : (c + 1) * 128])
            dec_t = [consts.tile([128, D], F32, name=f"dec_t{c}") for c in range(FC)]
            for p in range(D // 128):
                for c in range(FC):
                    pe_t(dec_t[c][:, p * 128 : (p + 1) * 128],
                         dec_nat[p][:, c * 128 : (c + 1) * 128])

            # ---- fc projections (full fp32 precision) ----
            # fc_enc_T[m] = [f_out chunk m, e] = W_enc @ enc.T + b_enc
            fc_enc_t = [consts.tile([128, E], F32, name=f"fc_enc_t{m}") for m in range(FC)]
            fc_dec_t = [consts.tile([128, D], F32, name=f"fc_dec_t{m}") for m in range(FC)]
            for m in range(FC):
                pf = ps.tile([128, E], F32, name="pf", tag="ps")
                for k in range(FC):
                    nc.tensor.matmul(
                        pf,
                        lhsT=w_enc_t[k][:, m * 128 : (m + 1) * 128],
                        rhs=enc_t[k],
                        start=(k == 0),
                        stop=(k == FC - 1),
                    )
                nc.vector.tensor_scalar_add(out=fc_enc_t[m], in0=pf, scalar1=b_enc_sb[m])
            for m in range(FC):
                pf = ps.tile([128, D], F32, name="pf2", tag="ps")
                for k in range(FC):
                    nc.tensor.matmul(
                        pf,
                        lhsT=w_dec_t[k][:, m * 128 : (m + 1) * 128],
                        rhs=dec_t[k],
                        start=(k == 0),
                        stop=(k == FC - 1),
                    )
                nc.vector.tensor_scalar_add(out=fc_dec_t[m], in0=pf, scalar1=b_dec_sb[m])

            # ---- v stationary tiles: 32 tiles of [128, 32], v in column r ----
            # tile r occupies cols [r*32, r*32+32); its column r is abs col r*33.
            # f32r tiles must be produced by f32r-rounding ops (tensor_scalar),
            # not Memset/Copy, to satisfy the BIR verifier + ISA checks.
            vw = []
            for c in range(FC):
                vt = consts.tile([128, 1024], F32R, name=f"vw{c}")
                for h in range(2):
                    nc.vector.tensor_scalar_mul(
                        out=vt[:, h * 512 : (h + 1) * 512], in0=enc_t[0], scalar1=0.0
                    )
                nc.vector.tensor_scalar_add(
                    out=vt[:, 0:1024:33], in0=vt[:, 0:32], scalar1=v_sb[c]
                )
                vw.append(vt)

            # ---- main loop ----
            # f32r matmuls require dst partition offset 0, so scores for each
            # 32-row quadrant of a d-block accumulate into their own [32, 512]
            # PSUM tile (4 per block).
            for blk in [b for _ in range(reps) for b in range(D // DBLK)]:
                pq = [None] * 4
                for g in range(N_GROUPS):
                    xg = [
                        xp.tile([128, G * E], F32R, name=f"x{c}", tag=f"x{c}")
                        for c in range(FC)
                    ]
                    for r in range(G):
                        dg = blk * DBLK + g * G + r
                        for c in range(FC):
                            nc.vector.tensor_scalar_add(
                                out=xg[c][:, r * E : (r + 1) * E],
                                in0=fc_enc_t[c],
                                scalar1=fc_dec_t[c][:, dg : dg + 1],
                            )
                    for c in range(FC):
                        nc.scalar.activation(out=xg[c], in_=xg[c], func=_AF.Tanh)
                    for r in range(G):
                        dl = g * G + r
                        j, r32 = dl // 32, dl % 32
                        if r32 == 0 and pq[j] is None:
                            pq[j] = ps.tile(
                                [32, E], F32, name=f"pq{j}", tag="psq", bufs=4
                            )
                        for c in range(FC):
                            nc.tensor.matmul(
                                pq[j],
                                lhsT=vw[c][:, r32 * 32 : (r32 + 1) * 32],
                                rhs=xg[c][:, r * E : (r + 1) * E],
                                start=(r32 == 0 and c == 0),
                                stop=(r32 == 31 and c == FC - 1),
                                tile_position=(0, 0),
                            )
                # per-quadrant softmax over e (each quadrant = 32 decoder rows)
                attn_q = []
                for j in range(4):
                    negmx = sm.tile([32, 1], F32, name="negmx", tag="negmx")
                    nc.vector.reduce_max(
                        out=negmx, in_=pq[j], axis=mybir.AxisListType.X, negate=True
                    )
                    et = sm.tile([32, E], F32, name="et", tag="et")
                    nc.scalar.activation(out=et, in_=pq[j], func=_AF.Exp, bias=negmx)
                    ssum = sm.tile([32, 1], F32, name="ssum", tag="ssum")
                    nc.vector.reduce_sum(out=ssum, in_=et, axis=mybir.AxisListType.X)
                    rs = sm.tile([32, 1], F32, name="rs", tag="rs")
                    nc.vector.reciprocal(rs, ssum)
                    aq = sm.tile([32, E], F32, name="aq", tag=f"aq{j}")
                    nc.vector.tensor_scalar_mul(out=aq, in0=et, scalar1=rs)
                    nc.sync.dma_start(
                        out=attn_o[blk * DBLK + j * 32 : blk * DBLK + (j + 1) * 32, :],
                        in_=aq,
                    )
                    attn_q.append(aq)
                # context = attn @ enc, contraction over e.
                # Transpose each [32 d, 128 e] piece to [128 e, 32 d] (fp32
                # transposes may write any free offset; dst partition is 0).
                pc = pctx.tile([128, F], F32, name="pc")
                for k in range(EC):
                    pt2 = ptr.tile([128, 128], F32, name="pt2", tag="pt")
                    for j in range(4):
                        nc.tensor.transpose(
                            pt2[:, j * 32 : (j + 1) * 32],
                            attn_q[j][:, k * 128 : (k + 1) * 128],
                            ident[0:32, 0:32],
                        )
                    at = sm.tile([128, 128], F32, name="at", tag="at")
                    nc.vector.tensor_copy(out=at, in_=pt2)
                    nc.tensor.matmul(
                        pc,
                        lhsT=at,
                        rhs=enc_nat[k],
                        start=(k == 0),
                        stop=(k == EC - 1),
                    )
                ctx_sb = sm.tile([128, F], F32, name="ctx_sb")
                nc.vector.tensor_copy(out=ctx_sb, in_=pc)
                nc.sync.dma_start(
                    out=ctx_o[blk * DBLK : (blk + 1) * DBLK, :], in_=ctx_sb
                )

    nc.compile()
    return nc


_NC = None


def _get_nc():
    global _NC
    if _NC is None:
        _NC = _build_nc()
    return _NC


def _in_maps(inputs):
    dec = np.asarray(inputs["decoder_states"], dtype=np.float32)
    enc = np.asarray(inputs["encoder_states"], dtype=np.float32)
    w_enc = np.asarray(inputs["W_enc"], dtype=np.float32)
    b_enc = np.asarray(inputs["b_enc"], dtype=np.float32)
    w_dec = np.asarray(inputs["W_dec"], dtype=np.float32)
    b_dec = np.asarray(inputs["b_dec"], dtype=np.float32)
    v = np.asarray(inputs["v"], dtype=np.float32)
    return [
        {
            "decoder_states": np.ascontiguousarray(dec[b]),
            "encoder_states": np.ascontiguousarray(enc[b]),
            "W_enc": w_enc,
            "b_enc": b_enc,
            "W_dec": w_dec,
            "b_dec": b_dec,
            "v": v,
        }
        for b in range(B)
    ]


def kernel(**inputs):
    nc = _get_nc()
    res = run_bass_kernel_spmd(
        nc, _in_maps(inputs), core_ids=list(range(N_CORES))
    ).results
    context = np.stack([r["context"] for r in res], axis=0)
    attn = np.stack([r["attn"] for r in res], axis=0)
    return context, attn
